# revision 1
# baseline (speedup 1.0000x reference)
"""v2 fused DQ+Add+LayerNorm+Quant kernel for TRN2, 8-core row-sharded.

Numerics (scheme A): host packs pk=[fp16(res) | fp16(int)] (ints exact in
fp16), device computes x in fp16 (xf32=0) or f32 (xf32=1); LN tail in f32;
q via the DVE f32->int8 RNE+saturating output converter.

Per 128-row tile:
  P1  (DVE): x = 0.01*pk_int + pk_res  [accum sum(x)]
  SQ  (ACT): sq = x^2                  [PSUM scratch, accum sum(x^2)]
  MEANS(ACT): means = sums * (1/DIM)
  NEGV(DVE): negvar = mu*mu - ex2      [small]
  STD (ACT): std = sqrt(eps - negvar)
  RSTD(DVE): rstd = 1/std              [small]
  D   (DVE): t = (x - mu) * w          [f32]
  E   (DVE): q = (t * rstd) + b        [int8 out]
Software-pipelined: loads lead P1 by 2 tiles, small stats one tile behind,
LN tail `lag` tiles behind P1.
"""

import numpy as np

import concourse.bacc as bacc
import concourse.bass as bass
import concourse.mybir as mybir
import concourse.tile as tile

TOKENS, DIM = 16384, 4096
N_CORES = 8
ROWS = TOKENS // N_CORES
P = 128
EPS = 1e-5

F32 = mybir.dt.float32
F16 = mybir.dt.float16
I8 = mybir.dt.int8
Alu = mybir.AluOpType
Act = mybir.ActivationFunctionType


def build_v2(rows: int = ROWS, repeats: int = 1, ring3_x: int = 1,
             io_bufs: int = 4, lag: int = 3, xf32: int = 0, ring3_q: int = 0,
             tpsum: int = 1, sp_dma: int = 0):
    nc = bacc.Bacc("TRN2", target_bir_lowering=False, debug=False)

    pk = nc.dram_tensor("pk", [rows, 2 * DIM], F16, kind="ExternalInput").ap()
    w = nc.dram_tensor("weight", [DIM], F32, kind="ExternalInput").ap()
    b = nc.dram_tensor("bias", [DIM], F32, kind="ExternalInput").ap()
    XDT = F32 if xf32 else F16
    x_out = nc.dram_tensor("x_out", [rows, DIM], XDT, kind="ExternalOutput").ap()
    q_out = nc.dram_tensor("q_out", [rows, DIM], I8, kind="ExternalOutput").ap()

    ntiles = rows // P
    n = ntiles * repeats

    with tile.TileContext(nc) as tc:
        with (
            tc.tile_pool(name="singles", bufs=1) as singles,
            tc.tile_pool(name="io", bufs=io_bufs) as io,
            tc.tile_pool(name="xh", bufs=lag + 2) as xhp,
            tc.tile_pool(name="work", bufs=2) as work,
            tc.tile_pool(name="sq", bufs=1,
                         space="SBUF" if tpsum else "PSUM") as sqp,
            tc.tile_pool(name="tp", bufs=1, space="PSUM") as tpp,
            tc.tile_pool(name="stats", bufs=lag + 2) as stats,
        ):
            wB = singles.tile([P, DIM], F32)
            bB = singles.tile([P, DIM], F32)
            nc.gpsimd.dma_start(out=wB, in_=bass.AP(
                tensor=w.tensor, offset=w.offset, ap=[[0, P], w.ap[0]]))
            nc.gpsimd.dma_start(out=bB, in_=bass.AP(
                tensor=b.tensor, offset=b.offset, ap=[[0, P], b.ap[0]]))
            eps_t = singles.tile([P, 1], F32)
            nc.vector.memset(eps_t, EPS)

            tiles = {}

            def loads(i):
                r0 = (i % ntiles) * P
                pt = io.tile([P, 2 * DIM], F16, tag="pk")
                if sp_dma:
                    ld = nc.sync
                else:
                    ld = nc.scalar if i % 2 == 0 else nc.sync
                ld.dma_start(out=pt, in_=pk[r0:r0 + P, :])
                tiles[("pk", i)] = pt

            def head_a(i):
                r0 = (i % ntiles) * P
                pt = tiles.pop(("pk", i))
                xh = xhp.tile([P, DIM], XDT, tag="x")
                sums = stats.tile([P, 2], F32, tag="sums")
                nc.vector.scalar_tensor_tensor(
                    out=xh, in0=pt[:, DIM:], scalar=0.01, in1=pt[:, 0:DIM],
                    op0=Alu.mult, op1=Alu.add, accum_out=sums[:, 0:1])
                if ring3_x:
                    nc.gpsimd.dma_start(out=x_out[r0:r0 + P, :], in_=xh)
                else:
                    st = nc.sync if i % 2 == 0 else nc.scalar
                    st.dma_start(out=x_out[r0:r0 + P, :], in_=xh)
                sq = sqp.tile([P, DIM], F32, tag="sq")
                nc.scalar.activation(out=sq, in_=xh, func=Act.Square,
                                     accum_out=sums[:, 1:2])
                means = stats.tile([P, 2], F32, tag="means")
                nc.scalar.activation(out=means, in_=sums, func=Act.Copy,
                                     scale=1.0 / DIM)
                tiles[("xh", i)] = xh
                tiles[("mu", i)] = means

            def head_b(i):
                means = tiles[("mu", i)]
                negvar = stats.tile([P, 1], F32, tag="negvar")
                nc.vector.tensor_scalar(
                    out=negvar, in0=means[:, 0:1], scalar1=means[:, 0:1],
                    scalar2=means[:, 1:2], op0=Alu.mult, op1=Alu.subtract)
                std = stats.tile([P, 1], F32, tag="std")
                nc.scalar.activation(out=std, in_=negvar, func=Act.Sqrt,
                                     bias=eps_t, scale=-1.0)
                rstd = stats.tile([P, 1], F32, tag="rstd")
                nc.vector.reciprocal(out=rstd, in_=std)
                tiles[("rstd", i)] = rstd

            def tail(i):
                r0 = (i % ntiles) * P
                xh = tiles.pop(("xh", i))
                mu = tiles.pop(("mu", i))[:, 0:1]
                rstd = tiles.pop(("rstd", i))
                if tpsum:
                    t = tpp.tile([P, DIM], F32, tag="t")
                else:
                    t = work.tile([P, DIM], F32, tag="t")
                qt = work.tile([P, DIM], I8, tag="q")
                nc.vector.scalar_tensor_tensor(
                    out=t, in0=xh, scalar=mu, in1=wB,
                    op0=Alu.subtract, op1=Alu.mult)
                nc.vector.scalar_tensor_tensor(
                    out=qt, in0=t, scalar=rstd, in1=bB,
                    op0=Alu.mult, op1=Alu.add)
                if ring3_q:
                    nc.gpsimd.dma_start(out=q_out[r0:r0 + P, :], in_=qt)
                elif sp_dma:
                    nc.sync.dma_start(out=q_out[r0:r0 + P, :], in_=qt)
                else:
                    st = nc.scalar if i % 2 == 0 else nc.sync
                    st.dma_start(out=q_out[r0:r0 + P, :], in_=qt)

            # software-pipelined schedule:
            #   step j: loads(j), head_a(j-2), head_b(j-3), tail(j-2-lag)
            assert lag >= 1
            for j in range(n + 2 + lag):
                if j < n:
                    loads(j)
                if 0 <= j - 2 < n:
                    head_a(j - 2)
                if 0 <= j - 3 < n:
                    head_b(j - 3)
                if 0 <= j - 2 - lag < n:
                    tail(j - 2 - lag)

    nc.finalize()
    return nc


_NC_CACHE = {}


def _get_nc(**kw):
    key = tuple(sorted(kw.items()))
    if key not in _NC_CACHE:
        _NC_CACHE[key] = build_v2(**kw)
    return _NC_CACHE[key]


def pack_inputs(residual_input_fp, input_int32):
    res = np.asarray(residual_input_fp, dtype=np.float32)
    qin = np.asarray(input_int32, dtype=np.int32)
    pk = np.empty((res.shape[0], 2 * DIM), np.float16)
    pk[:, :DIM] = res.astype(np.float16)
    pk[:, DIM:] = qin.astype(np.float16)  # |int| <= 2048: exact in fp16
    return pk


def kernel(residual_input_fp, input_int32, weight, bias,
           ring3_x=1, io_bufs=4, lag=3, xf32=0, ring3_q=0, tpsum=1, sp_dma=0):
    from concourse.bass_utils import run_bass_kernel_spmd
    import os

    pk = pack_inputs(residual_input_fp, input_int32)
    w = np.ascontiguousarray(np.asarray(weight, dtype=np.float32))
    b = np.ascontiguousarray(np.asarray(bias, dtype=np.float32))

    nc = _get_nc(rows=ROWS, ring3_x=ring3_x, io_bufs=io_bufs, lag=lag,
                 xf32=xf32, ring3_q=ring3_q, tpsum=tpsum, sp_dma=sp_dma)
    in_maps = []
    for c in range(N_CORES):
        sl = slice(c * ROWS, (c + 1) * ROWS)
        in_maps.append({"pk": pk[sl], "weight": w, "bias": b})

    try:
        out = run_bass_kernel_spmd(nc, in_maps, core_ids=list(range(N_CORES)))
    except ModuleNotFoundError:
        os.environ["BASS_NEVER_TRACE"] = "1"
        out = run_bass_kernel_spmd(nc, in_maps, core_ids=list(range(N_CORES)))
    x = np.concatenate([r["x_out"] for r in out.results], axis=0).astype(np.float32)
    q = np.concatenate([r["q_out"] for r in out.results], axis=0)
    return x, q


# test.py compatibility surface
DEFAULT_MODE = "v2"


def build_bass(rows: int = ROWS, repeats: int = 1, mode: str = DEFAULT_MODE):
    return build_v2(rows=rows, repeats=repeats)

